# revision 19
# baseline (speedup 1.0000x reference)
# Bass/Tile TRN2 kernel for nn_Attn_2130303779132 (general-score attention).
#
# Math: reference computes
#   proj = einsum('sbh,kh->sbk', enc, W) + b        # (S,B,H) huge matmul
#   energies[b,s] = <hidden[b], proj[s,b]>          # (B,S)
#   out = softmax(energies, axis=-1)
# Algebraically:
#   energies[b,s] = sum_h enc[s,b,h] * v[b,h] + (hidden[b]·bias)
# with v = hidden @ W.  The bias term is constant across s, so softmax
# removes it exactly.  The kernel computes v (tiny matmul), a batched
# dot over H against the streamed encoder outputs, and a softmax over
# S — memory bound on reading enc once.
#
# Sharding: data-parallel over batch. 8 cores x 2 batches each; no
# collectives.
#
# DMA layout (the whole point of this version): the per-core enc slice
# (S, 2, H) is contiguous in DRAM, so it is streamed as 4 chunks of
# 8MB where partition p holds JR=8 *consecutive* s-rows = one fully
# contiguous 64KB run — the SDMA max descriptor size.  512 descriptors
# total for enc (vs ~4096 with per-s-row tiling), spread over the
# SP/PE/ACT/DVE HWDGE rings so several DMA queues run in parallel.
# W and hidden^T are host-packed into one (128, 8208) tensor so the
# whole prologue is a single 128x32.8KB-descriptor DMA and needs no
# on-device transposes (no identity matrix load).
# The output is dumped in compute layout (128, 64) with one DMA and
# unshuffled on the host (pure index permutation, part of unsharding).

import numpy as np

import concourse.bacc as bacc
import concourse.bass as bass
import concourse.bass_isa as bass_isa
import concourse.tile as tile
from concourse import library_config, mybir
from concourse.bass_utils import run_bass_kernel_spmd

S, B, H = 4096, 16, 1024
NCORES = 8
BL = B // NCORES          # local batches per core = 2
P = 128                   # partitions
JR = 4                    # consecutive s-rows per partition per chunk
NCHK = S // (P * JR)      # 8 chunks of 512 s
NCOL = NCHK * JR          # 32 energy columns per batch
KR = H // P               # 8 W-rows per partition in the packed tensor
HCOL = KR * BL            # 16 packed hidden^T columns (first)
WFREE = HCOL + KR * H     # + 8192 packed W columns
WHSPL = HCOL + (KR // 2) * H   # split point: hidt + W rows r=0..3
# j-split of each chunk's 8 s-rows into sub-DMAs; the last chunk is
# split 6+2 so only 4 multiply-accum jobs remain after its final sub-DMA
SPLITS = ((4, 4), (4, 4), (4, 4), (4, 4))
F32 = mybir.dt.float32
ENC_BUFS = 3              # enc chunk buffers in flight
WARMUP_MM = 8             # PE pstate warmup matmuls before the v chain

# Ring schedule for the enc half-chunk DMAs, indices into
# [sync(SP), scalar(ACT), gpsimd(Pool SWDGE)] — the only engines that
# can initiate DMAs.  Early chunks lean on SP/Pool while ACT finishes
# its W half.
RINGS = (
    (1, 2),   # c0: ACT, Pool
    (0, 1),   # c1: SP, ACT
    (2, 1),   # c2: Pool, ACT
    (0, 2),   # c3: SP, Pool
)


def build_bass(loop_n: int = 1) -> bass.Bass:
    """loop_n > 1 wraps the kernel body in an on-device For loop —
    used only for steady-state timing (amortizes RPC/launch overhead)."""
    nc = bacc.Bacc("TRN2", target_bir_lowering=False, debug=False,
                   num_devices=NCORES)

    enc = nc.dram_tensor("enc", (S, BL, H), F32, kind="ExternalInput").ap()
    wh = nc.dram_tensor("wh", (P, WFREE), F32, kind="ExternalInput").ap()
    selc = nc.dram_tensor("selc", (BL, BL * P), F32,
                          kind="ExternalInput").ap()
    out = nc.dram_tensor("out", (P, BL * NCOL), F32, kind="ExternalOutput").ap()

    with tile.TileContext(nc) as tc:
        with (
            tc.tile_pool(name="consts", bufs=1) as consts,
            tc.tile_pool(name="encpool", bufs=ENC_BUFS) as encpool,
            tc.tile_pool(name="scratch", bufs=2) as scratch,
            tc.tile_pool(name="small", bufs=2) as small,
            tc.tile_pool(name="psumv", bufs=1, space="PSUM") as psumv,
            tc.tile_pool(name="psums", bufs=1, space="PSUM") as psums,
        ):
            pools = (consts, encpool, scratch, small, psumv, psums)

            def body():
                build_body(nc, pools, enc, wh, selc, out)

            if loop_n == 1:
                body()
            else:
                with tc.For_i(0, loop_n, 1):
                    body()

    nc.compile()
    return nc


def build_body(nc, pools, enc, wh, selc, out):
    consts, encpool, scratch, small, psumv, psums = pools
    ENG = [nc.sync, nc.scalar, nc.gpsimd]

    # Q7 library for partition_all_reduce, paid up front under the DMAs.
    nc.gpsimd.load_library(library_config.mlp)

    # ---- prologue loads: packed [hidden^T | W] as ONE DMA, first on the
    # otherwise-empty SP ring so it owns the DMA engines before the enc
    # stream starts; selector on ACT ----
    wh_sb = consts.tile([P, WFREE], F32, tag="wh")
    nc.sync.dma_start(out=wh_sb, in_=wh)
    selc_sb = consts.tile([BL, BL * P], F32, tag="selc")
    nc.scalar.dma_start(out=selc_sb, in_=selc)

    # ---- enc stream triggers for the first ENC_BUFS chunks ----
    enc_r = enc.rearrange("(c p j) b h -> c p j b h", p=P, j=JR)
    ets = []

    def issue_chunk(c):
        et = encpool.tile([P, JR, BL, H], F32, tag="enc", name=f"et{c}")
        ets.append(et)
        ENG[RINGS[c]].dma_start(out=et, in_=enc_r[c])

    for c0 in range(ENC_BUFS):
        issue_chunk(c0)

    # ones vectors for the cross-partition sum / broadcast matmuls
    ones_col = consts.tile([P, 1], F32, tag="ones_col")
    nc.vector.memset(ones_col, 1.0)
    ones_row = consts.tile([1, P], F32, tag="ones_row")
    nc.vector.memset(ones_row, 1.0)

    # ---- PE warmup: keep the tensor engine busy until the W tiles land
    # so the v chain below is costed/clocked at full pstate (the PE clock
    # ramps only after ~3us of continuous work).  Garbage results into a
    # scratch PSUM bank; no one reads them. ----
    warm_in = consts.tile([P, 512], F32, tag="warm")
    nc.vector.memset(warm_in, 0.0)
    psum_warm = psumv.tile([1, 512], F32, tag="warm", name="psum_warm")
    for _ in range(WARMUP_MM):
        nc.tensor.matmul(out=psum_warm, lhsT=ones_col, rhs=warm_in,
                         start=True, stop=True)

    # ---- v = hidden @ W  (PE, contraction over k on partitions) ----
    # wh_sb columns: [2r + b] = hidden[b, 8p+r]; [HCOL + r*H + h] = W[8p+r, h]
    psum_v = psumv.tile([BL, H], F32, tag="v")
    for n2 in range(0, H, 512):
        for r in range(KR):
            nc.tensor.matmul(
                out=psum_v[:, n2:n2 + 512],
                lhsT=wh_sb[:, BL * r: BL * r + BL],
                rhs=wh_sb[:, HCOL + r * H + n2: HCOL + r * H + n2 + 512],
                start=(r == 0),
                stop=(r == KR - 1),
            )
    v_sb = consts.tile([BL, H], F32, tag="vsb")
    nc.scalar.copy(out=v_sb, in_=psum_v)

    # ---- broadcast v rows to all partitions via selector matmul ----
    vb = consts.tile([P, BL, H], F32, tag="vb")
    for b in range(BL):
        psum_vb = psums.tile([P, H], F32, tag="vbp", name=f"psum_vb{b}")
        for n2 in range(0, H, 512):
            nc.tensor.matmul(
                out=psum_vb[:, n2:n2 + 512],
                lhsT=selc_sb[:, b * P:(b + 1) * P],
                rhs=v_sb[:, n2:n2 + 512],
                start=True,
                stop=True,
            )
        nc.scalar.copy(out=vb[:, b, :], in_=psum_vb)

    # ---- main loop: E2[p, b*32 + c*8 + j] = <enc[s], v[b]>, s=c*1024+8p+j
    # (fused multiply + free-dim accumulate on the DVE; TensorScalarPtr is
    # not a legal Pool-engine opcode on real TRN2, so no GPSIMD offload)
    E2 = consts.tile([P, BL * NCOL], F32, tag="E2")
    for c in range(NCHK):
        et = ets[c]
        for j in range(JR):
            for b in range(BL):
                idx = b * NCOL + c * JR + j
                prod = scratch.tile([P, H], F32, tag="prod")
                nc.vector.scalar_tensor_tensor(
                    out=prod, in0=et[:, j, b, :], scalar=1.0,
                    in1=vb[:, b, :],
                    op0=mybir.AluOpType.mult, op1=mybir.AluOpType.mult,
                    accum_out=E2[:, idx:idx + 1],
                )
        if c + ENC_BUFS < NCHK:
            issue_chunk(c + ENC_BUFS)

    # ---- softmax over all S per batch ----
    # per-partition max, then exact cross-partition max on gpsimd
    m2 = small.tile([P, BL], F32, tag="m2")
    for b in range(BL):
        nc.vector.tensor_reduce(
            out=m2[:, b:b + 1], in_=E2[:, b * NCOL:(b + 1) * NCOL],
            axis=mybir.AxisListType.X, op=mybir.AluOpType.max,
        )
    mall = small.tile([P, BL], F32, tag="mall")
    nc.gpsimd.partition_all_reduce(
        out_ap=mall, in_ap=m2, channels=P, reduce_op=bass_isa.ReduceOp.max,
    )
    negm = small.tile([P, BL], F32, tag="negm")
    nc.vector.tensor_scalar_mul(out=negm, in0=mall, scalar1=-1.0)

    eexp = small.tile([P, BL * NCOL], F32, tag="eexp")
    for b in range(BL):
        nc.scalar.activation(
            out=eexp[:, b * NCOL:(b + 1) * NCOL],
            in_=E2[:, b * NCOL:(b + 1) * NCOL],
            func=mybir.ActivationFunctionType.Exp,
            bias=negm[:, b:b + 1], scale=1.0,
        )

    rsum = small.tile([P, BL], F32, tag="rsum")
    for b in range(BL):
        nc.vector.tensor_reduce(
            out=rsum[:, b:b + 1], in_=eexp[:, b * NCOL:(b + 1) * NCOL],
            axis=mybir.AxisListType.X, op=mybir.AluOpType.add,
        )

    # cross-partition sum on PE: tot[b] = sum_p rsum[p, b]
    psum_tot = psums.tile([BL, 1], F32, tag="tot", name="psum_tot")
    nc.tensor.matmul(out=psum_tot, lhsT=rsum, rhs=ones_col,
                     start=True, stop=True)
    rt1 = small.tile([BL, 1], F32, tag="rt1")
    nc.vector.reciprocal(out=rt1, in_=psum_tot)

    # (2,1) -> (1,2) using the 2x2 identity hiding inside selc, then
    # broadcast to all partitions with a K=1 ones matmul
    psum_rt = psums.tile([1, BL], F32, tag="rtT", name="psum_rt")
    nc.tensor.transpose(out=psum_rt, in_=rt1,
                        identity=selc_sb[:, 0:BL * P:P])
    rt_sb = small.tile([1, BL], F32, tag="rtsb")
    nc.scalar.copy(out=rt_sb, in_=psum_rt)
    psum_rb = psums.tile([P, BL], F32, tag="rb", name="psum_rb")
    nc.tensor.matmul(out=psum_rb, lhsT=ones_row, rhs=rt_sb,
                     start=True, stop=True)
    rb_sb = small.tile([P, BL], F32, tag="rbsb")
    nc.scalar.copy(out=rb_sb, in_=psum_rb)

    probs = small.tile([P, BL * NCOL], F32, tag="probs")
    for b in range(BL):
        nc.vector.tensor_scalar_mul(
            out=probs[:, b * NCOL:(b + 1) * NCOL],
            in0=eexp[:, b * NCOL:(b + 1) * NCOL],
            scalar1=rb_sb[:, b:b + 1],
        )

    # raw layout dump; host unshuffles (p, b, c, j) -> s order
    nc.sync.dma_start(out=out, in_=probs)


_NC_CACHE = None


def _get_nc() -> bass.Bass:
    global _NC_CACHE
    if _NC_CACHE is None:
        _NC_CACHE = build_bass()
    return _NC_CACHE


def make_in_maps(hidden, encoder_outputs, W):
    hidden = np.asarray(hidden, dtype=np.float32)
    encoder_outputs = np.asarray(encoder_outputs, dtype=np.float32)
    W = np.ascontiguousarray(np.asarray(W, dtype=np.float32))
    wpack = W.reshape(P, KR * H)  # row p = W[8p:8p+8, :] flattened
    selc = np.zeros((BL, BL * P), dtype=np.float32)
    for b in range(BL):
        selc[b, b * P:(b + 1) * P] = 1.0
    in_maps = []
    for c in range(NCORES):
        hid_local = hidden[0, c * BL:(c + 1) * BL, :]          # (2, 1024)
        hidt = hid_local.T.reshape(P, KR * BL)                 # [p, 2r+b]
        wh = np.ascontiguousarray(
            np.concatenate([hidt, wpack], axis=1))             # (128, 8208)
        in_maps.append(
            {
                "enc": np.ascontiguousarray(
                    encoder_outputs[:, c * BL:(c + 1) * BL, :]
                ),
                "wh": wh,
                "selc": selc,
            }
        )
    return in_maps


def unshuffle_out(raw):
    """(128, 64) compute-layout dump -> (BL, S); s = c*1024 + 8p + j."""
    return (
        np.asarray(raw)
        .reshape(P, BL, NCHK, JR)
        .transpose(1, 2, 0, 3)
        .reshape(BL, S)
    )


def kernel(hidden, encoder_outputs, W, b, **run_kwargs):
    # `b` (the nn.Linear bias) shifts every energy row by a per-batch
    # constant, which softmax cancels exactly — unused on device.
    nc = _get_nc()
    in_maps = make_in_maps(hidden, encoder_outputs, W)
    res = run_bass_kernel_spmd(
        nc, in_maps, core_ids=list(range(NCORES)), **run_kwargs
    )
    outs = [unshuffle_out(r["out"]) for r in res.results]
    full = np.concatenate(outs, axis=0)  # (16, 4096)
    return full.reshape(B, 1, S).astype(np.float32)


# revision 28
# speedup vs baseline: 1.0987x; 1.0987x over previous
# Bass/Tile TRN2 kernel for nn_Attn_2130303779132 (general-score attention).
#
# Math: reference computes
#   proj = einsum('sbh,kh->sbk', enc, W) + b        # (S,B,H) huge matmul
#   energies[b,s] = <hidden[b], proj[s,b]>          # (B,S)
#   out = softmax(energies, axis=-1)
# Algebraically:
#   energies[b,s] = sum_h enc[s,b,h] * v[b,h] + (hidden[b]·bias)
# with v = hidden @ W.  The bias term is constant across s, so softmax
# removes it exactly.  The kernel computes v (tiny matmul), a batched
# dot over H against the streamed encoder outputs, and a softmax over
# S — memory bound on reading enc once.
#
# Sharding: data-parallel over batch. 8 cores x 2 batches each; no
# collectives.
#
# DMA layout (the whole point of this version): the per-core enc slice
# (S, 2, H) is contiguous in DRAM, so it is streamed as 8 chunks of
# 4MB where partition p holds JR=4 *consecutive* s-rows = one fully
# contiguous 32KB descriptor run (vs 8KB runs with per-s-row tiling,
# ~4x fewer descriptors).  Chunks alternate between the SP and ACT
# HWDGE rings so both hardware DMA queues stream in parallel, with
# four chunk buffers in flight.  W and hidden^T are host-packed into
# one (128, 8208) tensor so the whole prologue is a single
# 128x32.8KB-descriptor DMA and needs no on-device transposes (no
# identity matrix load).  A short PE warmup chain keeps the tensor
# engine clocked up before the v matmuls.  The output is dumped in
# compute layout (128, 64) with one DMA and unshuffled on the host
# (pure index permutation, part of unsharding).

import numpy as np

import concourse.bacc as bacc
import concourse.bass as bass
import concourse.bass_isa as bass_isa
import concourse.tile as tile
from concourse import library_config, mybir
from concourse.bass_utils import run_bass_kernel_spmd

S, B, H = 4096, 16, 1024
NCORES = 8
BL = B // NCORES          # local batches per core = 2
P = 128                   # partitions
JR = 4                    # consecutive s-rows per partition per chunk
NCHK = S // (P * JR)      # 8 chunks of 512 s
NCOL = NCHK * JR          # 32 energy columns per batch
KR = H // P               # 8 W-rows per partition in the packed tensor
HCOL = KR * BL            # 16 packed hidden^T columns (first)
WFREE = HCOL + KR * H     # + 8192 packed W columns
WHSPL = HCOL + (KR // 2) * H   # split point: hidt + W rows r=0..3
F32 = mybir.dt.float32
ENC_BUFS = 4              # enc chunk buffers in flight
WARMUP_MM = 8             # PE pstate warmup matmuls before the v chain

# Ring for each chunk's single DMA, indices into [sync(SP), scalar(ACT)]
# — the two HWDGE rings.  GPSIMD's software-DGE ring is deliberately NOT
# used for the enc stream: its completion semantics raced the consumer
# on real hardware (intermittent NaN), so Pool only runs the softmax
# all-reduce.  SP also carries the big W load, so ACT leads.
RINGS = (1, 0, 1, 0, 1, 0, 1, 0)


def build_bass(loop_n: int = 1) -> bass.Bass:
    """loop_n > 1 wraps the kernel body in an on-device For loop —
    used only for steady-state timing (amortizes RPC/launch overhead)."""
    nc = bacc.Bacc("TRN2", target_bir_lowering=False, debug=False,
                   num_devices=NCORES)

    enc = nc.dram_tensor("enc", (S, BL, H), F32, kind="ExternalInput").ap()
    wh = nc.dram_tensor("wh", (P, WFREE), F32, kind="ExternalInput").ap()
    selc = nc.dram_tensor("selc", (BL, BL * P), F32,
                          kind="ExternalInput").ap()
    out = nc.dram_tensor("out", (P, BL * NCOL), F32,
                         kind="ExternalOutput").ap()

    with tile.TileContext(nc) as tc:
        with (
            tc.tile_pool(name="consts", bufs=1) as consts,
            tc.tile_pool(name="encpool", bufs=ENC_BUFS) as encpool,
            tc.tile_pool(name="scratch", bufs=2) as scratch,
            tc.tile_pool(name="small", bufs=2) as small,
            tc.tile_pool(name="psumv", bufs=1, space="PSUM") as psumv,
            tc.tile_pool(name="psums", bufs=1, space="PSUM") as psums,
        ):
            pools = (consts, encpool, scratch, small, psumv, psums)

            def body():
                build_body(nc, pools, enc, wh, selc, out)

            if loop_n == 1:
                body()
            else:
                with tc.For_i(0, loop_n, 1):
                    body()

    nc.compile()
    return nc


def build_body(nc, pools, enc, wh, selc, out):
    consts, encpool, scratch, small, psumv, psums = pools
    ENG = [nc.sync, nc.scalar, nc.gpsimd]

    # Q7 library for partition_all_reduce, paid up front under the DMAs.
    nc.gpsimd.load_library(library_config.mlp)

    # ---- prologue loads: packed [hidden^T | W] as ONE DMA, first on the
    # otherwise-empty SP ring so it owns the DMA engines before the enc
    # stream starts; selector on ACT ----
    wh_sb = consts.tile([P, WFREE], F32, tag="wh")
    nc.sync.dma_start(out=wh_sb, in_=wh)
    selc_sb = consts.tile([BL, BL * P], F32, tag="selc")
    nc.scalar.dma_start(out=selc_sb, in_=selc)

    # ---- enc stream triggers for the first ENC_BUFS chunks ----
    enc_r = enc.rearrange("(c p j) b h -> c p j b h", p=P, j=JR)
    ets = []

    def issue_chunk(c):
        et = encpool.tile([P, JR, BL, H], F32, tag="enc", name=f"et{c}")
        ets.append(et)
        ENG[RINGS[c]].dma_start(out=et, in_=enc_r[c])

    for c0 in range(ENC_BUFS):
        issue_chunk(c0)

    # ones vectors for the cross-partition sum / broadcast matmuls
    ones_col = consts.tile([P, 1], F32, tag="ones_col")
    nc.vector.memset(ones_col, 1.0)
    ones_row = consts.tile([1, P], F32, tag="ones_row")
    nc.vector.memset(ones_row, 1.0)

    # ---- PE warmup: keep the tensor engine busy until the W tiles land
    # so the v chain below is costed/clocked at full pstate (the PE clock
    # ramps only after ~3us of continuous work).  Garbage results into a
    # scratch PSUM bank; no one reads them. ----
    warm_in = consts.tile([P, 512], F32, tag="warm")
    nc.vector.memset(warm_in, 0.0)
    psum_warm = psumv.tile([1, 512], F32, tag="warm", name="psum_warm")
    for _ in range(WARMUP_MM):
        nc.tensor.matmul(out=psum_warm, lhsT=ones_col, rhs=warm_in,
                         start=True, stop=True)

    # ---- v = hidden @ W  (PE, contraction over k on partitions) ----
    # wh_sb columns: [2r + b] = hidden[b, 8p+r]; [HCOL + r*H + h] = W[8p+r, h]
    psum_v = psumv.tile([BL, H], F32, tag="v")
    for n2 in range(0, H, 512):
        for r in range(KR):
            nc.tensor.matmul(
                out=psum_v[:, n2:n2 + 512],
                lhsT=wh_sb[:, BL * r: BL * r + BL],
                rhs=wh_sb[:, HCOL + r * H + n2: HCOL + r * H + n2 + 512],
                start=(r == 0),
                stop=(r == KR - 1),
            )
    v_sb = consts.tile([BL, H], F32, tag="vsb")
    nc.scalar.copy(out=v_sb, in_=psum_v)

    # ---- broadcast v rows to all partitions via selector matmul ----
    vb = consts.tile([P, BL, H], F32, tag="vb")
    for b in range(BL):
        psum_vb = psums.tile([P, H], F32, tag="vbp", name=f"psum_vb{b}")
        for n2 in range(0, H, 512):
            nc.tensor.matmul(
                out=psum_vb[:, n2:n2 + 512],
                lhsT=selc_sb[:, b * P:(b + 1) * P],
                rhs=v_sb[:, n2:n2 + 512],
                start=True,
                stop=True,
            )
        nc.scalar.copy(out=vb[:, b, :], in_=psum_vb)

    # ---- main loop: E2[p, b*32 + c*4 + j] = <enc[s], v[b]>, s=c*512+4p+j
    # (fused multiply + free-dim accumulate on the DVE; TensorScalarPtr is
    # not a legal Pool-engine opcode on real TRN2, so no GPSIMD offload)
    E2 = consts.tile([P, BL * NCOL], F32, tag="E2")
    for c in range(NCHK):
        et = ets[c]
        for j in range(JR):
            for b in range(BL):
                idx = b * NCOL + c * JR + j
                prod = scratch.tile([P, H], F32, tag="prod")
                nc.vector.scalar_tensor_tensor(
                    out=prod, in0=et[:, j, b, :], scalar=1.0,
                    in1=vb[:, b, :],
                    op0=mybir.AluOpType.mult, op1=mybir.AluOpType.mult,
                    accum_out=E2[:, idx:idx + 1],
                )
        if c + ENC_BUFS < NCHK:
            issue_chunk(c + ENC_BUFS)

    # ---- softmax over all S per batch ----
    # per-partition max, then exact cross-partition max on gpsimd
    m2 = small.tile([P, BL], F32, tag="m2")
    nc.vector.tensor_reduce(
        out=m2, in_=E2.rearrange("p (b k) -> p b k", b=BL),
        axis=mybir.AxisListType.X, op=mybir.AluOpType.max,
    )
    mall = small.tile([P, BL], F32, tag="mall")
    nc.gpsimd.partition_all_reduce(
        out_ap=mall, in_ap=m2, channels=P, reduce_op=bass_isa.ReduceOp.max,
    )
    negm = small.tile([P, BL], F32, tag="negm")
    nc.vector.tensor_scalar_mul(out=negm, in0=mall, scalar1=-1.0)

    eexp = small.tile([P, BL * NCOL], F32, tag="eexp")
    for b in range(BL):
        nc.scalar.activation(
            out=eexp[:, b * NCOL:(b + 1) * NCOL],
            in_=E2[:, b * NCOL:(b + 1) * NCOL],
            func=mybir.ActivationFunctionType.Exp,
            bias=negm[:, b:b + 1], scale=1.0,
        )

    rsum = small.tile([P, BL], F32, tag="rsum")
    nc.vector.tensor_reduce(
        out=rsum, in_=eexp.rearrange("p (b k) -> p b k", b=BL),
        axis=mybir.AxisListType.X, op=mybir.AluOpType.add,
    )

    # cross-partition sum on PE: tot[b] = sum_p rsum[p, b]
    psum_tot = psums.tile([BL, 1], F32, tag="tot", name="psum_tot")
    nc.tensor.matmul(out=psum_tot, lhsT=rsum, rhs=ones_col,
                     start=True, stop=True)
    rt1 = small.tile([BL, 1], F32, tag="rt1")
    nc.vector.reciprocal(out=rt1, in_=psum_tot)

    # (2,1) -> (1,2) using the 2x2 identity hiding inside selc, then
    # broadcast to all partitions with a K=1 ones matmul
    psum_rt = psums.tile([1, BL], F32, tag="rtT", name="psum_rt")
    nc.tensor.transpose(out=psum_rt, in_=rt1,
                        identity=selc_sb[:, 0:BL * P:P])
    rt_sb = small.tile([1, BL], F32, tag="rtsb")
    nc.scalar.copy(out=rt_sb, in_=psum_rt)
    psum_rb = psums.tile([P, BL], F32, tag="rb", name="psum_rb")
    nc.tensor.matmul(out=psum_rb, lhsT=ones_row, rhs=rt_sb,
                     start=True, stop=True)
    rb_sb = small.tile([P, BL], F32, tag="rbsb")
    nc.scalar.copy(out=rb_sb, in_=psum_rb)

    probs = small.tile([P, BL * NCOL], F32, tag="probs")
    for b in range(BL):
        nc.vector.tensor_scalar_mul(
            out=probs[:, b * NCOL:(b + 1) * NCOL],
            in0=eexp[:, b * NCOL:(b + 1) * NCOL],
            scalar1=rb_sb[:, b:b + 1],
        )

    # raw layout dump; host unshuffles (p, b, c, j) -> s order.
    # ACT ring: SP already carries the W load plus half the enc stream.
    nc.scalar.dma_start(out=out, in_=probs)


_NC_CACHE = None


def _get_nc() -> bass.Bass:
    global _NC_CACHE
    if _NC_CACHE is None:
        _NC_CACHE = build_bass()
    return _NC_CACHE


def make_in_maps(hidden, encoder_outputs, W):
    hidden = np.asarray(hidden, dtype=np.float32)
    encoder_outputs = np.asarray(encoder_outputs, dtype=np.float32)
    W = np.ascontiguousarray(np.asarray(W, dtype=np.float32))
    wpack = W.reshape(P, KR * H)  # row p = W[8p:8p+8, :] flattened
    selc = np.zeros((BL, BL * P), dtype=np.float32)
    for b in range(BL):
        selc[b, b * P:(b + 1) * P] = 1.0
    in_maps = []
    for c in range(NCORES):
        hid_local = hidden[0, c * BL:(c + 1) * BL, :]          # (2, 1024)
        hidt = hid_local.T.reshape(P, KR * BL)                 # [p, 2r+b]
        wh = np.ascontiguousarray(
            np.concatenate([hidt, wpack], axis=1))             # (128, 8208)
        in_maps.append(
            {
                "enc": np.ascontiguousarray(
                    encoder_outputs[:, c * BL:(c + 1) * BL, :]
                ),
                "wh": wh,
                "selc": selc,
            }
        )
    return in_maps


def unshuffle_out(raw):
    """(128, 64) compute-layout dump -> (BL, S); s = c*P*JR + p*JR + j."""
    return (
        np.asarray(raw)
        .reshape(P, BL, NCHK, JR)
        .transpose(1, 2, 0, 3)
        .reshape(BL, S)
    )


def kernel(hidden, encoder_outputs, W, b, **run_kwargs):
    # `b` (the nn.Linear bias) shifts every energy row by a per-batch
    # constant, which softmax cancels exactly — unused on device.
    nc = _get_nc()
    in_maps = make_in_maps(hidden, encoder_outputs, W)
    res = run_bass_kernel_spmd(
        nc, in_maps, core_ids=list(range(NCORES)), **run_kwargs
    )
    outs = [unshuffle_out(r["out"]) for r in res.results]
    full = np.concatenate(outs, axis=0)  # (16, 4096)
    return full.reshape(B, 1, S).astype(np.float32)
